# revision 9
# baseline (speedup 1.0000x reference)
"""MoE top-2 routing kernel for 8 NeuronCores (Trainium2, Bass/Tile).

Strategy: expert-parallel. Each of the 8 cores owns one expert's weights.
The host computes the router top-2 *dispatch decision* (which tokens go to
which expert), gathers each expert's tokens into a capacity-padded buffer,
and each core then computes -- fully on device -- the fp32 router
logits/top-2/softmax for its own tokens plus the bf16 SwiGLU expert MLP,
scaling its output by its own combine weight. The host scatter-adds the
8 partial outputs into the full [T, D] result.

Problem shape (hardcoded): T=2048, D=1024, F=4096, E=8, TOPK=2.
"""

import sys

sys.path.insert(0, "/opt/trn_rl_repo")

import numpy as np
import ml_dtypes

import concourse.bass as bass
import concourse.mybir as mybir
import concourse.tile as tile
from concourse import bacc
from concourse.bass_utils import run_bass_kernel_spmd

T, D, F, E, TOPK = 2048, 1024, 4096, 8, 2
P = 128
C = 640           # per-expert token capacity (max observed load 551 + margin)
KD = D // P       # 8  contraction chunks over D
FT = F // P       # 32 tiles over F
CT = C // P       # 5  token tiles
NEG_BIG = -1e30

f32 = mybir.dt.float32
bf16 = mybir.dt.bfloat16


def _c_chunks(total, step=512):
    out = []
    c0 = 0
    while c0 < total:
        cn = min(step, total - c0)
        out.append((c0, cn))
        c0 += cn
    return out


def build_nc(use_silu=True):
    # use_silu=False replaces the fused Silu activation (not implemented in
    # CoreSim) with sigmoid + an extra multiply; numerically equivalent.
    # Bacc (not plain Bass): its finalize() runs the sync-legalization
    # passes (move_matmul_waits_to_ldweights, generate_event_semaphores)
    # that TRN2's 1-wait-per-instruction constraint requires.
    nc = bacc.Bacc()

    xt = nc.declare_dram_parameter("xt", [D, C], f32, isOutput=False)
    rw = nc.declare_dram_parameter("rw", [D, E], f32, isOutput=False)
    wg = nc.declare_dram_parameter("wg", [D, F], bf16, isOutput=False)
    wu = nc.declare_dram_parameter("wu", [D, F], bf16, isOutput=False)
    wd = nc.declare_dram_parameter("wd", [F, D], bf16, isOutput=False)
    y = nc.declare_dram_parameter("y", [C, D], f32, isOutput=True)

    xt_r = xt.rearrange("(k p) c -> p k c", p=P)      # [P, KD, C]
    rw_r = rw.rearrange("(k p) e -> p k e", p=P)      # [P, KD, E]
    wg_r = wg.rearrange("(k p) f -> p k f", p=P)      # [P, KD, F]
    wu_r = wu.rearrange("(k p) f -> p k f", p=P)
    wd_r = wd.rearrange("(t p) d -> p t d", p=P)      # [P, FT, D]
    y_r = y.rearrange("(i p) d -> p i d", p=P)        # [P, CT, D]

    with tile.TileContext(nc) as tc:
        with (
            tc.tile_pool(name="singles", bufs=1) as sb,
            tc.tile_pool(name="stream", bufs=3) as st,
            tc.tile_pool(name="rt", bufs=2) as rt,
            tc.tile_pool(name="psum_gu", bufs=2, space="PSUM") as pgu,
            tc.tile_pool(name="psum_y", bufs=2, space="PSUM") as py,
            tc.tile_pool(name="psum_r", bufs=2, space="PSUM") as pr,
        ):
            # ---- load x (fp32), make bf16 copy ----
            xt_sb = sb.tile([P, KD, C], f32)
            xtb = sb.tile([P, KD, C], bf16)
            for k in range(KD):
                nc.sync.dma_start(xt_sb[:, k], xt_r[:, k])
                nc.vector.tensor_copy(xtb[:, k], xt_sb[:, k])

            rw_sb = sb.tile([P, KD, E], f32)
            nc.sync.dma_start(rw_sb[:], rw_r[:])

            # ---- prefetch wd (used only in phase 2) ----
            wd_sb = sb.tile([P, FT, D], bf16)
            for t in range(0, FT, 4):
                nc.sync.dma_start(wd_sb[:, t : t + 4], wd_r[:, t : t + 4])

            # ---- on-device router: fp32 logits, top-2, softmax ----
            # rw columns are pre-rotated by the host so that THIS core's
            # expert is column 0.
            w_tiles = []
            for i in range(CT):
                lg_ps = pr.tile([P, E], f32, tag="lg")
                for k in range(KD):
                    nc.tensor.matmul(
                        lg_ps,
                        xt_sb[:, k, i * P : (i + 1) * P],
                        rw_sb[:, k],
                        start=(k == 0),
                        stop=(k == KD - 1),
                    )
                l = rt.tile([P, E], f32, tag="l")
                # not tensor_copy: DVE TensorCopy lowers to a TR-struct
                # encoding that only fits ONE sync wait (walrus NCC_INLA001)
                nc.vector.tensor_scalar_add(l, lg_ps, 0.0)
                m1 = rt.tile([P, 1], f32, tag="m1")
                nc.vector.tensor_reduce(m1, l, axis=mybir.AxisListType.X,
                                        op=mybir.AluOpType.max)
                # mask out the argmax, then max again for the 2nd largest
                eq1 = rt.tile([P, E], f32, tag="eq1")
                nc.vector.tensor_scalar(eq1, l, m1, None,
                                        mybir.AluOpType.is_equal)
                tmp = rt.tile([P, E], f32, tag="tmp")
                nc.vector.tensor_scalar_mul(tmp, eq1, NEG_BIG)
                nc.vector.tensor_add(tmp, tmp, l)
                m2 = rt.tile([P, 1], f32, tag="m2")
                nc.vector.tensor_reduce(m2, tmp, axis=mybir.AxisListType.X,
                                        op=mybir.AluOpType.max)
                # softmax over {m1, m2}: w1 = 1/(1+exp(m2-m1)), w2 = 1-w1
                dd = rt.tile([P, 1], f32, tag="dd")
                nc.vector.tensor_sub(dd, m2, m1)
                # keep exp() input in LUT range (pad rows give m2-m1 = -1e30)
                nc.vector.tensor_scalar_max(dd, dd, -80.0)
                ed = rt.tile([P, 1], f32, tag="ed")
                nc.scalar.activation(ed, dd, mybir.ActivationFunctionType.Exp)
                den = rt.tile([P, 1], f32, tag="den")
                nc.vector.tensor_scalar_add(den, ed, 1.0)
                inv = rt.tile([P, 1], f32, tag="inv")
                nc.vector.reciprocal(inv, den)
                e2 = rt.tile([P, 1], f32, tag="e2")
                nc.vector.tensor_mul(e2, ed, inv)
                # my expert's logit is column 0
                le = l[:, 0:1]
                q1 = rt.tile([P, 1], f32, tag="q1")
                nc.vector.tensor_tensor(q1, le, m1, mybir.AluOpType.is_equal)
                q2 = rt.tile([P, 1], f32, tag="q2")
                nc.vector.tensor_tensor(q2, le, m2, mybir.AluOpType.is_equal)
                nc.vector.tensor_mul(q1, q1, inv)
                nc.vector.tensor_mul(q2, q2, e2)
                w_i = sb.tile([P, 1], f32, tag=f"w{i}")
                nc.vector.tensor_add(w_i, q1, q2)
                w_tiles.append(w_i)

            # ---- phase 1: HT[f, c] = silu(Wg.T x) * (Wu.T x), bf16 ----
            ht = sb.tile([P, FT, C], bf16)
            for f in range(FT):
                wg_t = st.tile([P, KD, P], bf16, tag="wg")
                nc.sync.dma_start(wg_t[:], wg_r[:, :, f * P : (f + 1) * P])
                wu_t = st.tile([P, KD, P], bf16, tag="wu")
                nc.sync.dma_start(wu_t[:], wu_r[:, :, f * P : (f + 1) * P])
                for c0, cn in _c_chunks(C):
                    gt_ps = pgu.tile([P, 512], f32, tag="gt")
                    ut_ps = pgu.tile([P, 512], f32, tag="ut")
                    for k in range(KD):
                        nc.tensor.matmul(
                            gt_ps[:, :cn], wg_t[:, k], xtb[:, k, c0 : c0 + cn],
                            start=(k == 0), stop=(k == KD - 1),
                        )
                    for k in range(KD):
                        nc.tensor.matmul(
                            ut_ps[:, :cn], wu_t[:, k], xtb[:, k, c0 : c0 + cn],
                            start=(k == 0), stop=(k == KD - 1),
                        )
                    sg = st.tile([P, 512], f32, tag="sg")
                    if use_silu:
                        nc.scalar.activation(sg[:, :cn], gt_ps[:, :cn],
                                             mybir.ActivationFunctionType.Silu)
                    else:
                        nc.scalar.activation(sg[:, :cn], gt_ps[:, :cn],
                                             mybir.ActivationFunctionType.Sigmoid)
                        nc.vector.tensor_tensor(sg[:, :cn], sg[:, :cn],
                                                gt_ps[:, :cn], mybir.AluOpType.mult)
                    nc.vector.tensor_tensor(ht[:, f, c0 : c0 + cn], sg[:, :cn],
                                            ut_ps[:, :cn], mybir.AluOpType.mult)

            # ---- phase 2: y[c, d] = w[c] * (HT.T @ Wd) ----
            for i in range(CT):
                for d0, dn in _c_chunks(D):
                    y_ps = py.tile([P, 512], f32, tag="y")
                    for t in range(FT):
                        nc.tensor.matmul(
                            y_ps[:, :dn],
                            ht[:, t, i * P : (i + 1) * P],
                            wd_sb[:, t, d0 : d0 + dn],
                            start=(t == 0),
                            stop=(t == FT - 1),
                        )
                    o_sb = st.tile([P, 512], f32, tag="o")
                    nc.vector.tensor_scalar_mul(o_sb[:, :dn], y_ps[:, :dn],
                                                w_tiles[i])
                    nc.sync.dma_start(y_r[:, i, d0 : d0 + dn], o_sb[:, :dn])

    nc.finalize()
    return nc


_NC_CACHE = None


def _get_nc():
    global _NC_CACHE
    if _NC_CACHE is None:
        _NC_CACHE = build_nc()
    return _NC_CACHE


def kernel(x, router_w_DE, w_gate_EDF, w_up_EDF, w_down_EFD):
    x = np.asarray(x, dtype=np.float32)
    router_w_DE = np.asarray(router_w_DE, dtype=np.float32)

    # host-side dispatch decision (routing math is re-done on device)
    logits = x.astype(np.float64) @ router_w_DE.astype(np.float64)
    top2 = np.argsort(-logits, axis=1)[:, :TOPK]  # [T, 2]

    wg_b = np.asarray(w_gate_EDF, dtype=ml_dtypes.bfloat16)
    wu_b = np.asarray(w_up_EDF, dtype=ml_dtypes.bfloat16)
    wd_b = np.asarray(w_down_EFD, dtype=ml_dtypes.bfloat16)

    in_maps = []
    idx_lists = []
    for e in range(E):
        idx = np.nonzero((top2 == e).any(axis=1))[0]
        assert len(idx) <= C, f"expert {e} overflow: {len(idx)} > {C}"
        idx_lists.append(idx)
        xt_e = np.zeros((D, C), dtype=np.float32)
        xt_e[:, : len(idx)] = x[idx].T
        rw_e = np.roll(router_w_DE, -e, axis=1).copy()  # my expert -> col 0
        in_maps.append({
            "xt": xt_e,
            "rw": rw_e,
            "wg": np.ascontiguousarray(wg_b[e]),
            "wu": np.ascontiguousarray(wu_b[e]),
            "wd": np.ascontiguousarray(wd_b[e]),
        })

    nc = _get_nc()
    res = run_bass_kernel_spmd(nc, in_maps, list(range(E))).results

    out = np.zeros((T, D), dtype=np.float32)
    for e in range(E):
        idx = idx_lists[e]
        out[idx] += res[e]["y"][: len(idx)]
    return out


# revision 11
# speedup vs baseline: 1.0229x; 1.0229x over previous
"""MoE top-2 routing kernel for 8 NeuronCores (Trainium2, Bass/Tile).

Strategy: expert-parallel. Each of the 8 cores owns one expert's weights.
The host computes the router top-2 *dispatch decision* (which tokens go to
which expert), gathers each expert's tokens into a capacity-padded buffer,
and each core then computes -- fully on device -- the fp32 router
logits/top-2/softmax for its own tokens plus the fp16 SwiGLU expert MLP,
scaling its output by its own combine weight. The host scatter-adds the
8 partial outputs into the full [T, D] result.

Problem shape (hardcoded): T=2048, D=1024, F=4096, E=8, TOPK=2.
"""

import sys

sys.path.insert(0, "/opt/trn_rl_repo")

import numpy as np
import ml_dtypes

import concourse.bass as bass
import concourse.mybir as mybir
import concourse.tile as tile
from concourse import bacc
from concourse.bass_utils import run_bass_kernel_spmd

T, D, F, E, TOPK = 2048, 1024, 4096, 8, 2
P = 128
C = 640           # per-expert token capacity (max observed load 551 + margin)
KD = D // P       # 8  contraction chunks over D
FT = F // P       # 32 tiles over F
CT = C // P       # 5  token tiles
NEG_BIG = -1e30
N_WARMUP_MM = 14  # PE warmup matmuls during the initial DMA window (HAM)

f32 = mybir.dt.float32
f16 = mybir.dt.float16
NP16 = np.float16


def _c_chunks(total, step=512):
    out = []
    c0 = 0
    while c0 < total:
        cn = min(step, total - c0)
        out.append((c0, cn))
        c0 += cn
    return out


def build_nc(use_silu=True):
    # use_silu=False replaces the fused Silu activation (not implemented in
    # CoreSim) with sigmoid + an extra multiply; numerically equivalent.
    # Bacc (not plain Bass): its finalize() runs the sync-legalization
    # passes (move_matmul_waits_to_ldweights, generate_event_semaphores)
    # that TRN2's 1-wait-per-instruction constraint requires.
    nc = bacc.Bacc()

    xt = nc.declare_dram_parameter("xt", [D, C], f32, isOutput=False)
    rw = nc.declare_dram_parameter("rw", [D, E], f32, isOutput=False)
    wg = nc.declare_dram_parameter("wg", [D, F], f16, isOutput=False)
    wu = nc.declare_dram_parameter("wu", [D, F], f16, isOutput=False)
    wd = nc.declare_dram_parameter("wd", [F, D], f16, isOutput=False)
    y = nc.declare_dram_parameter("y", [C, D], f32, isOutput=True)

    xt_r = xt.rearrange("(k p) c -> p k c", p=P)      # [P, KD, C]
    rw_r = rw.rearrange("(k p) e -> p k e", p=P)      # [P, KD, E]
    wg_r = wg.rearrange("(k p) f -> p k f", p=P)      # [P, KD, F]
    wu_r = wu.rearrange("(k p) f -> p k f", p=P)
    wd_r = wd.rearrange("(t p) d -> p t d", p=P)      # [P, FT, D]
    y_r = y.rearrange("(i p) d -> p i d", p=P)        # [P, CT, D]

    with tile.TileContext(nc) as tc:
        with (
            tc.tile_pool(name="singles", bufs=1) as sb,
            tc.tile_pool(name="stream", bufs=3) as st,
            tc.tile_pool(name="rt", bufs=2) as rt,
            tc.tile_pool(name="psum_gu", bufs=2, space="PSUM") as pgu,
            tc.tile_pool(name="psum_y", bufs=2, space="PSUM") as py,
            tc.tile_pool(name="psum_r", bufs=2, space="PSUM") as pr,
        ):
            # ---- PE warmup: garbage matmuls on a memset tile so the HAM
            # clock-gate releases (1.2 -> 2.4 GHz) while the x DMA runs ----
            warm_sb = st.tile([P, 512], f16, tag="warm")
            nc.gpsimd.memset(warm_sb[:], 0.0)
            for _ in range(N_WARMUP_MM):
                warm_ps = py.tile([P, 512], f32, tag="y")
                nc.tensor.matmul(warm_ps, warm_sb[:, :P], warm_sb,
                                 start=True, stop=True)

            # ---- load x (fp32), make fp16 copy ----
            xt_sb = sb.tile([P, KD, C], f32)
            xtb = sb.tile([P, KD, C], f16)
            for k in range(KD):
                nc.sync.dma_start(xt_sb[:, k], xt_r[:, k])
                nc.vector.tensor_copy(xtb[:, k], xt_sb[:, k])

            # ---- phase 1: HT[f, c] = silu(Wg.T x) * (Wu.T x), fp16 ----
            wd_sb = sb.tile([P, FT, D], f16)
            ht = sb.tile([P, FT, C], f16)
            for f in range(FT):
                wg_t = st.tile([P, KD, P], f16, tag="wg")
                nc.sync.dma_start(wg_t[:], wg_r[:, :, f * P : (f + 1) * P])
                wu_t = st.tile([P, KD, P], f16, tag="wu")
                nc.sync.dma_start(wu_t[:], wu_r[:, :, f * P : (f + 1) * P])
                if f % 4 == 2:
                    # stagger the wd prefetch behind the early wg/wu loads
                    t = (f - 2) // 4 * 4
                    nc.sync.dma_start(wd_sb[:, t : t + 4], wd_r[:, t : t + 4])
                for c0, cn in _c_chunks(C):
                    gt_ps = pgu.tile([P, 512], f32, tag="gt")
                    ut_ps = pgu.tile([P, 512], f32, tag="ut")
                    for k in range(KD):
                        nc.tensor.matmul(
                            gt_ps[:, :cn], wg_t[:, k], xtb[:, k, c0 : c0 + cn],
                            start=(k == 0), stop=(k == KD - 1),
                        )
                    for k in range(KD):
                        nc.tensor.matmul(
                            ut_ps[:, :cn], wu_t[:, k], xtb[:, k, c0 : c0 + cn],
                            start=(k == 0), stop=(k == KD - 1),
                        )
                    sg = st.tile([P, 512], f32, tag="sg")
                    if use_silu:
                        nc.scalar.activation(sg[:, :cn], gt_ps[:, :cn],
                                             mybir.ActivationFunctionType.Silu)
                    else:
                        nc.scalar.activation(sg[:, :cn], gt_ps[:, :cn],
                                             mybir.ActivationFunctionType.Sigmoid)
                        nc.vector.tensor_tensor(sg[:, :cn], sg[:, :cn],
                                                gt_ps[:, :cn], mybir.AluOpType.mult)
                    nc.vector.tensor_tensor(ht[:, f, c0 : c0 + cn], sg[:, :cn],
                                            ut_ps[:, :cn], mybir.AluOpType.mult)

            # last wd chunk (loop above covers t = 0..27)
            nc.sync.dma_start(wd_sb[:, 28:32], wd_r[:, 28:32])

            # ---- on-device router: fp32 logits, top-2, softmax ----
            # Placed after phase 1 in program order so it runs while the PE
            # is busy (it only gates the phase-2 copy-out). rw columns are
            # pre-rotated by the host so THIS core's expert is column 0.
            rw_sb = sb.tile([P, KD, E], f32)
            nc.sync.dma_start(rw_sb[:], rw_r[:])
            w_tiles = []
            for i in range(CT):
                lg_ps = pr.tile([P, E], f32, tag="lg")
                for k in range(KD):
                    nc.tensor.matmul(
                        lg_ps,
                        xt_sb[:, k, i * P : (i + 1) * P],
                        rw_sb[:, k],
                        start=(k == 0),
                        stop=(k == KD - 1),
                    )
                l = rt.tile([P, E], f32, tag="l")
                # not tensor_copy: DVE TensorCopy lowers to a TR-struct
                # encoding that only fits ONE sync wait (walrus NCC_INLA001)
                nc.vector.tensor_scalar_add(l, lg_ps, 0.0)
                m1 = rt.tile([P, 1], f32, tag="m1")
                nc.vector.tensor_reduce(m1, l, axis=mybir.AxisListType.X,
                                        op=mybir.AluOpType.max)
                # mask out the argmax, then max again for the 2nd largest
                eq1 = rt.tile([P, E], f32, tag="eq1")
                nc.vector.tensor_scalar(eq1, l, m1, None,
                                        mybir.AluOpType.is_equal)
                tmp = rt.tile([P, E], f32, tag="tmp")
                nc.vector.tensor_scalar_mul(tmp, eq1, NEG_BIG)
                nc.vector.tensor_add(tmp, tmp, l)
                m2 = rt.tile([P, 1], f32, tag="m2")
                nc.vector.tensor_reduce(m2, tmp, axis=mybir.AxisListType.X,
                                        op=mybir.AluOpType.max)
                # softmax over {m1, m2}: w1 = 1/(1+exp(m2-m1)), w2 = 1-w1
                dd = rt.tile([P, 1], f32, tag="dd")
                nc.vector.tensor_sub(dd, m2, m1)
                # keep exp() input in LUT range (pad rows give m2-m1 = -1e30)
                nc.vector.tensor_scalar_max(dd, dd, -80.0)
                ed = rt.tile([P, 1], f32, tag="ed")
                nc.scalar.activation(ed, dd, mybir.ActivationFunctionType.Exp)
                den = rt.tile([P, 1], f32, tag="den")
                nc.vector.tensor_scalar_add(den, ed, 1.0)
                inv = rt.tile([P, 1], f32, tag="inv")
                nc.vector.reciprocal(inv, den)
                e2 = rt.tile([P, 1], f32, tag="e2")
                nc.vector.tensor_mul(e2, ed, inv)
                # my expert's logit is column 0
                le = l[:, 0:1]
                q1 = rt.tile([P, 1], f32, tag="q1")
                nc.vector.tensor_tensor(q1, le, m1, mybir.AluOpType.is_equal)
                q2 = rt.tile([P, 1], f32, tag="q2")
                nc.vector.tensor_tensor(q2, le, m2, mybir.AluOpType.is_equal)
                nc.vector.tensor_mul(q1, q1, inv)
                nc.vector.tensor_mul(q2, q2, e2)
                w_i = sb.tile([P, 1], f32, tag=f"w{i}")
                nc.vector.tensor_add(w_i, q1, q2)
                w_tiles.append(w_i)

            # ---- phase 2: y[c, d] = w[c] * (HT.T @ Wd) ----
            for i in range(CT):
                for d0, dn in _c_chunks(D):
                    y_ps = py.tile([P, 512], f32, tag="y")
                    for t in range(FT):
                        nc.tensor.matmul(
                            y_ps[:, :dn],
                            ht[:, t, i * P : (i + 1) * P],
                            wd_sb[:, t, d0 : d0 + dn],
                            start=(t == 0),
                            stop=(t == FT - 1),
                        )
                    o_sb = st.tile([P, 512], f32, tag="o")
                    nc.vector.tensor_scalar_mul(o_sb[:, :dn], y_ps[:, :dn],
                                                w_tiles[i])
                    nc.sync.dma_start(y_r[:, i, d0 : d0 + dn], o_sb[:, :dn])

    nc.finalize()
    return nc


_NC_CACHE = None


def _get_nc():
    global _NC_CACHE
    if _NC_CACHE is None:
        _NC_CACHE = build_nc()
    return _NC_CACHE


def make_in_maps(x, router_w_DE, w_gate_EDF, w_up_EDF, w_down_EFD):
    x = np.asarray(x, dtype=np.float32)
    router_w_DE = np.asarray(router_w_DE, dtype=np.float32)

    # host-side dispatch decision (routing math is re-done on device)
    logits = x.astype(np.float64) @ router_w_DE.astype(np.float64)
    top2 = np.argsort(-logits, axis=1)[:, :TOPK]  # [T, 2]

    wg_b = np.asarray(w_gate_EDF, dtype=NP16)
    wu_b = np.asarray(w_up_EDF, dtype=NP16)
    wd_b = np.asarray(w_down_EFD, dtype=NP16)

    in_maps = []
    idx_lists = []
    for e in range(E):
        idx = np.nonzero((top2 == e).any(axis=1))[0]
        assert len(idx) <= C, f"expert {e} overflow: {len(idx)} > {C}"
        idx_lists.append(idx)
        xt_e = np.zeros((D, C), dtype=np.float32)
        xt_e[:, : len(idx)] = x[idx].T
        rw_e = np.roll(router_w_DE, -e, axis=1).copy()  # my expert -> col 0
        in_maps.append({
            "xt": xt_e,
            "rw": rw_e,
            "wg": np.ascontiguousarray(wg_b[e]),
            "wu": np.ascontiguousarray(wu_b[e]),
            "wd": np.ascontiguousarray(wd_b[e]),
        })
    return in_maps, idx_lists


def kernel(x, router_w_DE, w_gate_EDF, w_up_EDF, w_down_EFD):
    in_maps, idx_lists = make_in_maps(
        x, router_w_DE, w_gate_EDF, w_up_EDF, w_down_EFD)

    nc = _get_nc()
    res = run_bass_kernel_spmd(nc, in_maps, list(range(E))).results

    out = np.zeros((T, D), dtype=np.float32)
    for e in range(E):
        idx = idx_lists[e]
        out[idx] += res[e]["y"][: len(idx)]
    return out
